# revision 36
# baseline (speedup 1.0000x reference)
"""GNN message-passing attention kernel for Trainium2 (Bass/Tile).

Problem: 3 iterations of masked single-head attention over 1024 independent
graphs (N=256 nodes, V=40 features, QK=50).

Sharding: data-parallel on the leading F axis -- 128 graphs per NeuronCore
across 8 cores.  Weights replicated.  Full inputs in, full output out.

End-to-end wall time is dominated by the host<->device tunnel (~30 MB/s), so
the wire format is minimized:
  - values cross as int8 with a per-node f16 scale (10.5 + 0.5 MB), decoded
    to f32 on device (the ones-column the kernel carries is memset there);
  - adjacency crosses bit-packed along the neighbor axis (uint8, 8.4 MB) and
    is unpacked on-device with DVE shift+and, then cast to bf16;
  - the additive softmax mask is produced on the PE by transposing the
    unpacked adjacency against a MASKC-scaled identity directly into the
    score PSUM (no host-side transpose or bf16 cast);
  - the output crosses back as int8 with a per-node f16 scale (10.5 + 0.5 MB;
    the DVE int8 cast rounds to nearest, so quant error <= rowmax/252) and
    is dequantized on host.
Host prep overlaps the wire: values go to jax.device_put (async) before the
adjacency packbits runs, so the pack cost hides under the values transfer.
The PJRT shard_map callable is built once and cached (the library path
re-traces jax.jit and re-concatenates inputs on every call); the previous
call's device-resident output is donated back as the next call's output
buffer so no zero-buffer crosses the wire.

Device dataflow per 2-graph pipeline step (8 interleaved streams), as in the
proven baseline: q/k biases ride a ones-row through the weight matmuls
(fp32r), one Tanh per q|k PSUM block, scores accumulate on top of the
PE-transposed mask, one Exp with bias -1000 produces transposed numerators,
nv comes straight off num^T, and a per-partition reciprocal normalizes during
the PSUM->SBUF move (the ones-column refreshes itself for free).
"""

import math
import sys

import numpy as np

sys.path.insert(0, "/opt/trn_rl_repo")

import concourse.bass as bass  # noqa: E402
import concourse.mybir as mybir  # noqa: E402
from concourse import bacc, tile  # noqa: E402
from concourse.masks import make_identity  # noqa: E402

# Problem constants (hardcoded per harness contract).
F, N, V, QK = 1024, 256, 40, 50
ITERS = 3
SCALE = math.sqrt(50.0)  # NUM_QK = 50
MASKC = 1000.0 * SCALE  # adj * MASKC accumulated into e; exp bias -1000
N_CORES = 8
G = F // N_CORES  # graphs per core
NC2 = N // 128  # 2 partition chunks of the node axis
PB = N // 8  # packed bytes per adjacency row

F32 = mybir.dt.float32
F32R = mybir.dt.float32r  # fp32 data through the fast (replicated) PE path
F16 = mybir.dt.float16
BF16 = mybir.dt.bfloat16
U8 = mybir.dt.uint8
I8 = mybir.dt.int8
QMAX = 126.0  # int8 quant guard band (<127 so rounding can't wrap)

DEFAULT_BUFS = dict(io=10, work=10, small=11, vnb=22, vnhb=10, adjpb=6,
                    a8b=6, vnob=10, pmain=3, paux=2)


def build_nc(g_count=G, gb=2, streams=8, group=4, bufs=None):
    """Build the single-core Bass program (SPMD across 8 cores)."""
    B = dict(DEFAULT_BUFS)
    if bufs:
        B.update(bufs)
    streams = min(streams, g_count // gb)
    assert g_count % (gb * streams) == 0
    group = min(group, streams)
    nc = bacc.Bacc("TRN2", target_bir_lowering=False, debug=False)

    values_d = nc.dram_tensor("values", [g_count, N, V], I8, kind="ExternalInput")
    vscale_d = nc.dram_tensor("vscale", [g_count, N], F16, kind="ExternalInput")
    adjp_d = nc.dram_tensor("adjp", [g_count, N, PB], U8, kind="ExternalInput")
    wq_d = nc.dram_tensor("wq_aug", [V + 1, QK], F32R, kind="ExternalInput")
    wk_d = nc.dram_tensor("wk_aug", [V + 1, QK], F32R, kind="ExternalInput")
    out_d = nc.dram_tensor("out", [g_count, N, V], I8, kind="ExternalOutput")
    scales_d = nc.dram_tensor("scales", [g_count, N], F16, kind="ExternalOutput")

    with tile.TileContext(nc) as tc:
        with (
            tc.tile_pool(name="const", bufs=1) as constp,
            tc.tile_pool(name="io", bufs=B["io"]) as iop,
            tc.tile_pool(name="work", bufs=B["work"]) as workp,
            tc.tile_pool(name="small", bufs=B["small"]) as smallp,
            tc.tile_pool(name="pmain", bufs=B["pmain"], space="PSUM") as pmainp,
            tc.tile_pool(name="paux", bufs=B["paux"], space="PSUM") as pauxp,
        ):
            wq_sb = constp.tile([V + 1, QK], F32R)
            nc.sync.dma_start(wq_sb, wq_d[:, :])
            wk_sb = constp.tile([V + 1, QK], F32R)
            nc.sync.dma_start(wk_sb, wk_d[:, :])
            expbias_sb = constp.tile([128, 1], F32)
            nc.gpsimd.memset(expbias_sb, -1000.0)
            id_f32 = constp.tile([128, 128], F32)
            make_identity(nc, id_f32)
            # MASKC-scaled identity: moving operand that turns the unpacked
            # adjacency into the additive mask during the PE transpose.
            idm_mask = constp.tile([128, 128], BF16)
            nc.gpsimd.memset(idm_mask, 0.0)
            nc.gpsimd.affine_select(
                out=idm_mask, in_=idm_mask,
                compare_op=mybir.AluOpType.not_equal,
                fill=MASKC, base=0, pattern=[[-1, 128]], channel_multiplier=1,
            )

            class Stream:
                pass

            def phase_load(st, g0):
                st.prev_g0 = getattr(st, "g0", None)
                st.prev_vno = getattr(st, "vno", None)
                st.prev_sc = getattr(st, "sc", None)
                st.g0 = g0
                gsl = slice(g0, g0 + gb)
                st.vnh = iop.tile([128, gb, NC2, V], I8, tag="vnh",
                                  bufs=B["vnhb"])
                nc.sync.dma_start(
                    st.vnh,
                    values_d[gsl, :, :].rearrange("g (c p) v -> p g c v", c=NC2),
                )
                st.vsh = iop.tile([128, gb, NC2], F16, tag="vsh",
                                  bufs=B["vnhb"])
                nc.sync.dma_start(
                    st.vsh, vscale_d[gsl, :].rearrange("g (c p) -> p g c", c=NC2)
                )
                st.adjp = iop.tile([128, gb * NC2 * PB], U8, tag="adjp",
                                   bufs=B["adjpb"])
                nc.sync.dma_start(
                    st.adjp.rearrange("p (g c b) -> p g c b", g=gb, c=NC2),
                    adjp_d[gsl, :, :].rearrange("g (c p) b -> p g c b", c=NC2),
                )

            def phase_castv(st):
                vs32 = smallp.tile([128, gb, NC2], F32, tag="vs32")
                nc.vector.tensor_copy(vs32, st.vsh)
                st.vn = iop.tile([128, gb, NC2, V + 1], F32, tag="vn",
                                 bufs=B["vnb"])
                for g in range(gb):
                    for jc in range(NC2):
                        nc.vector.tensor_scalar_mul(
                            st.vn[:, g, jc, 0:V],
                            st.vnh[:, g, jc, :],
                            vs32[:, g, jc : jc + 1],
                        )
                nc.gpsimd.memset(st.vn[:, :, :, V : V + 1], 1.0)
                st.vnh = None
                st.vsh = None

            def phase_unpack(st):
                # adj[g, j, l] bits, l = 8*b + k (packbits MSB-first).
                a8 = smallp.tile([128, gb * NC2 * PB, 8], U8, tag="a8",
                                 bufs=B["a8b"])
                for k in range(8):
                    nc.vector.tensor_scalar(
                        a8[:, :, k], st.adjp, 7 - k, 1,
                        mybir.AluOpType.logical_shift_right,
                        mybir.AluOpType.bitwise_and,
                    )
                st.adjw = iop.tile([128, gb, NC2, N], BF16, tag="adjw")
                nc.vector.tensor_copy(
                    st.adjw.rearrange("p g c l -> p (g c l)"),
                    a8.rearrange("p x k -> p (x k)"),
                )
                st.adjp = None

            def phase_vt0(st):
                psum_vt = pauxp.tile([V + 1, gb * N], F32, tag="paux")
                for g in range(gb):
                    for c in range(NC2):
                        nc.tensor.transpose(
                            psum_vt[:, N * g + 128 * c : N * g + 128 * (c + 1)],
                            st.vn[:, g, c, :],
                            id_f32,
                        )
                st.vt = smallp.tile([V + 1, gb * N], F32R, tag="vt")
                nc.vector.tensor_copy(st.vt, psum_vt)

            def phase_qk(st):
                # [50, (qk-half, g, j)]: q in bank 0, k in bank 1.
                # Bias rides the vt ones-row (weights row V).
                st.psum_qk = pmainp.tile([QK, 2 * gb * N], F32, tag="pmain")
                nc.tensor.matmul(st.psum_qk[:, 0 : gb * N], wq_sb, st.vt)
                nc.tensor.matmul(st.psum_qk[:, gb * N : 2 * gb * N], wk_sb, st.vt)

            def phase_tanh(st):
                st.qk = workp.tile([QK, 2 * gb * N], F32R, tag="qk")
                nc.scalar.activation(
                    st.qk, st.psum_qk, mybir.ActivationFunctionType.Tanh
                )
                st.psum_qk = None

            def phase_et(st):
                st.psum_e = pmainp.tile([128, gb, NC2 * N], F32, tag="pmain", name="pe")
                # Scores init each 256-col region (start=True), then the two
                # 128-col mask transposes accumulate inside it -- the
                # start=True write must cover a superset of the start=False
                # writes or the scheduler can reorder them.
                for g in range(gb):
                    for lc in range(NC2):
                        nc.tensor.matmul(
                            st.psum_e[:, g, N * lc : N * (lc + 1)],
                            st.qk[:, gb * N + N * g + 128 * lc : gb * N + N * g + 128 * (lc + 1)],
                            st.qk[:, N * g : N * (g + 1)],
                            start=True,
                            stop=False,
                            skip_group_check=True,
                        )
                        for jc in range(NC2):
                            nc.tensor.matmul(
                                st.psum_e[:, g, N * lc + 128 * jc : N * lc + 128 * (jc + 1)],
                                st.adjw[:, g, jc, 128 * lc : 128 * (lc + 1)],
                                idm_mask,
                                start=False,
                                stop=(jc == NC2 - 1),
                                skip_group_check=True,
                            )

            def phase_exp(st):
                st.numt = workp.tile([128, gb, NC2 * N], F32, tag="numt")
                nc.scalar.activation(
                    st.numt,
                    st.psum_e,
                    mybir.ActivationFunctionType.Exp,
                    bias=expbias_sb,
                    scale=1.0 / SCALE,
                )
                st.psum_e = None

            def phase_nv(st):
                # nv[j, v] = sum_l num[j, l] v[l, v], directly off numT
                # (l already on partitions); the vn ones-column makes col V
                # the softmax row-sum.
                st.psum_nv = pauxp.tile([128, gb, NC2, V + 1], F32, tag="paux")
                for g in range(gb):
                    for jc in range(NC2):
                        for lc in range(NC2):
                            nc.tensor.matmul(
                                st.psum_nv[:, g, jc, :],
                                st.numt[:, g, N * lc + 128 * jc : N * lc + 128 * jc + 128],
                                st.vn[:, g, lc, :],
                                start=(lc == 0),
                                stop=(lc == NC2 - 1),
                            )
                st.numt = None

            def phase_norm(st, last):
                recip = smallp.tile([128, gb, NC2], F32, tag="recip")
                nc.vector.reciprocal(recip, st.psum_nv[:, :, :, V])
                if last:
                    # Final iteration: int8 row quantization straight off the
                    # unnormalized PSUM (the softmax recip folds into the
                    # scale, q = psum * QMAX/rowmax, v' = q * rowmax*recip/QMAX).
                    rm = smallp.tile([128, gb, NC2], F32, tag="rm")
                    nc.vector.tensor_reduce(
                        rm, st.psum_nv[:, :, :, 0:V],
                        axis=mybir.AxisListType.X, op=mybir.AluOpType.max,
                        apply_absolute_value=True,
                    )
                    qs = smallp.tile([128, gb, NC2], F32, tag="qs")
                    nc.vector.reciprocal(qs, rm)
                    st.sc = iop.tile([128, gb, NC2], F16, tag="sc",
                                     bufs=B["vnob"])
                    nc.vector.tensor_mul(st.sc, rm, recip)
                    st.vno = iop.tile([128, gb, NC2, V], I8, tag="vno",
                                      bufs=B["vnob"])
                    for g in range(gb):
                        for jc in range(NC2):
                            nc.vector.tensor_scalar(
                                st.vno[:, g, jc, :],
                                st.psum_nv[:, g, jc, 0:V],
                                qs[:, g, jc : jc + 1],
                                QMAX,
                                mybir.AluOpType.mult,
                                mybir.AluOpType.mult,
                            )
                else:
                    st.vn = iop.tile([128, gb, NC2, V + 1], F32, tag="vn",
                                     bufs=B["vnb"])
                    for g in range(gb):
                        for jc in range(NC2):
                            nc.vector.tensor_scalar_mul(
                                st.vn[:, g, jc, :],
                                st.psum_nv[:, g, jc, :],
                                recip[:, g, jc : jc + 1],
                            )
                st.psum_nv = None

            def phase_vt(st):
                psum_vt = pauxp.tile([V + 1, gb * N], F32, tag="paux")
                for g in range(gb):
                    for jc in range(NC2):
                        nc.tensor.transpose(
                            psum_vt[:, N * g + 128 * jc : N * g + 128 * (jc + 1)],
                            st.vn[:, g, jc, :],
                            id_f32,
                        )
                st.vt = smallp.tile([V + 1, gb * N], F32R, tag="vt")
                nc.vector.tensor_copy(st.vt, psum_vt)

            def phase_store_prev(st):
                # SWDGE (gpsimd) queue: keeps result stores out of the SP
                # FIFO so the next round's loads always prefetch early.
                gsl = slice(st.prev_g0, st.prev_g0 + gb)
                nc.gpsimd.dma_start(
                    out_d[gsl, :, :].rearrange("g (c p) v -> p g c v", c=NC2),
                    st.prev_vno,
                )
                nc.gpsimd.dma_start(
                    scales_d[gsl, :].rearrange("g (c p) -> p g c", c=NC2),
                    st.prev_sc,
                )

            sts = [Stream() for _ in range(streams)]
            for _i, _st in enumerate(sts):
                _st.sid = _i
            grps = [sts[i : i + group] for i in range(0, streams, group)]

            def run_iter(grp, t):
                for st in grp:
                    phase_qk(st)
                for st in grp:
                    phase_tanh(st)
                for st in grp:
                    phase_et(st)
                for st in grp:
                    phase_exp(st)
                for st in grp:
                    phase_nv(st)
                for st in grp:
                    phase_norm(st, t == ITERS - 1)
                if t < ITERS - 1:
                    for st in grp:
                        phase_vt(st)

            # Groups round-robin per iteration so one group's next phase
            # fills the pipeline while the other finishes; the previous
            # round's store and the next round's load ride inside the
            # rotation so round boundaries never resynchronize the streams.
            rounds = g_count // (gb * streams)
            for r in range(rounds):
                for grp in grps:
                    for st in grp:
                        phase_load(st, gb * (r * streams + st.sid))
                for grp in grps:
                    for st in grp:
                        if r > 0:
                            phase_store_prev(st)
                    for st in grp:
                        phase_castv(st)
                    for st in grp:
                        phase_unpack(st)
                    for st in grp:
                        phase_vt0(st)
                for t in range(ITERS):
                    for grp in grps:
                        run_iter(grp, t)
            for grp in grps:
                for st in grp:
                    st.prev_g0, st.prev_vno, st.prev_sc = st.g0, st.vno, st.sc
                    phase_store_prev(st)

    nc.compile()
    return nc


class _State:
    pass


_STATE = None


def _get_state():
    global _STATE
    if _STATE is None:
        import jax
        from jax.sharding import Mesh, NamedSharding, PartitionSpec
        from jax.experimental.shard_map import shard_map
        from concourse.bass2jax import (
            _bass_exec_p,
            install_neuronx_cc_hook,
            partition_id_tensor,
        )

        install_neuronx_cc_hook()
        nc = build_nc()
        assert nc.dbg_addr is None

        partition_name = (
            nc.partition_id_tensor.name if nc.partition_id_tensor else None
        )
        in_names, out_names, out_avals = [], [], []
        for alloc in nc.m.functions[0].allocations:
            if not isinstance(alloc, mybir.MemoryLocationSet):
                continue
            name = alloc.memorylocations[0].name
            if alloc.kind == "ExternalInput":
                if name != partition_name:
                    in_names.append(name)
            elif alloc.kind == "ExternalOutput":
                out_avals.append(
                    jax.core.ShapedArray(
                        tuple(alloc.tensor_shape), mybir.dt.np(alloc.dtype)
                    )
                )
                out_names.append(name)
        n_params = len(in_names)
        in_names_all = in_names + out_names
        if partition_name is not None:
            in_names_all.append(partition_name)

        def _body(*args):
            operands = list(args)
            if partition_name is not None:
                operands.append(partition_id_tensor())
            outs = _bass_exec_p.bind(
                *operands,
                out_avals=tuple(out_avals),
                in_names=tuple(in_names_all),
                out_names=tuple(out_names),
                lowering_input_output_aliases=(),
                sim_require_finite=True,
                sim_require_nnan=True,
                nc=nc,
            )
            return tuple(outs)

        devices = jax.devices()[:N_CORES]
        assert len(devices) == N_CORES
        mesh = Mesh(np.asarray(devices), ("core",))
        sharding = NamedSharding(mesh, PartitionSpec("core"))
        n_outs = len(out_names)
        donate = tuple(range(n_params, n_params + n_outs))
        sharded = jax.jit(
            shard_map(
                _body,
                mesh=mesh,
                in_specs=(PartitionSpec("core"),) * (n_params + n_outs),
                out_specs=(PartitionSpec("core"),) * n_outs,
                check_rep=False,
            ),
            donate_argnums=donate,
            keep_unused=True,
        )
        out_shapes = [
            ((N_CORES * a.shape[0], *a.shape[1:]), a.dtype) for a in out_avals
        ]

        def _zeros():
            import jax.numpy as jnp

            return tuple(jnp.zeros(s, d) for s, d in out_shapes)

        st = _State()
        st.jax = jax
        st.sharded = sharded
        st.in_names = in_names
        st.out_names = out_names
        st.sharding = sharding
        st.zeros_fn = jax.jit(_zeros, out_shardings=(sharding,) * n_outs)
        st.last_out = None
        _STATE = st
    return _STATE


def _aug(W, b):
    aug = np.zeros((V + 1, QK), np.float32)
    aug[0:V] = np.asarray(W, np.float32).T
    aug[V] = np.asarray(b, np.float32)
    return np.tile(aug, (N_CORES, 1))  # replicated across the core mesh


def _scratch(st):
    # Per-call numpy scratch, reused across calls (all device reads complete
    # before kernel() returns, so cross-call reuse cannot race a transfer).
    if not hasattr(st, "b_tmp"):
        st.b_tmp = np.empty((F, N, V), np.float32)
        st.b_s = np.empty((F, N), np.float32)
        st.b_s2 = np.empty((F, N), np.float32)
        st.b_vq = np.empty((F, N, V), np.int8)
        st.b_vs = np.empty((F, N), np.float16)
        st.b_a8 = np.empty((F, N, N), np.uint8)
    return st


def kernel(**inputs):
    st = _scratch(_get_state())
    # Stage values first (async H2D) so the adjacency packbits overlaps the
    # transfer's I/O waits (single-core host: the CPU share serializes).
    values = np.asarray(inputs["values"]).reshape(F, N, V)
    tmp, s = st.b_tmp, st.b_s
    # row abs-max via min+max (two reads, no 43MB abs write)
    s2 = st.b_s2
    np.max(values, axis=-1, out=s)
    np.min(values, axis=-1, out=s2)
    np.negative(s2, out=s2)
    np.maximum(s, s2, out=s)
    np.maximum(s, 1e-30, out=s)  # all-zero rows decode to 0 either way
    np.multiply(s, 1.0 / QMAX, out=s2)
    st.b_vs[...] = s2
    s_dev = st.jax.device_put(st.b_vs, st.sharding)
    np.divide(QMAX, s, out=s)
    np.multiply(values, s[:, :, None], out=tmp)
    np.rint(tmp, out=tmp)
    st.b_vq[...] = tmp
    v_dev = st.jax.device_put(st.b_vq, st.sharding)
    adj = np.asarray(inputs["adjacency_matrix"]).reshape(F, N, N)
    np.copyto(st.b_a8, adj, casting="unsafe")
    adjp = np.packbits(st.b_a8, axis=-1)
    a_dev = st.jax.device_put(adjp, st.sharding)
    wq_aug = _aug(inputs["Wq"], inputs["bq"])
    wk_aug = _aug(inputs["Wk"], inputs["bk"])
    wkey = hash((wq_aug.tobytes(), wk_aug.tobytes()))
    if getattr(st, "wkey", None) != wkey:
        st.wq_dev = st.jax.device_put(wq_aug, st.sharding)
        st.wk_dev = st.jax.device_put(wk_aug, st.sharding)
        st.wkey = wkey
    arrs = {
        "values": v_dev,
        "vscale": s_dev,
        "adjp": a_dev,
        "wq_aug": st.wq_dev,
        "wk_aug": st.wk_dev,
    }
    out_bufs = st.last_out if st.last_out is not None else st.zeros_fn()
    outs = st.sharded(*[arrs[n] for n in st.in_names], *out_bufs)
    st.last_out = outs  # device-resident; donated as next call's out buffers
    by_name = dict(zip(st.out_names, outs))
    by_name["out"].copy_to_host_async()
    by_name["scales"].copy_to_host_async()
    q = np.asarray(by_name["out"])  # (F, N, V) int8
    sc = np.asarray(by_name["scales"])  # (F, N) f16
    sc32 = st.b_s
    np.multiply(sc, 1.0 / QMAX, out=sc32, casting="unsafe")
    full = np.empty((F, N, V), np.float32)
    np.multiply(q, sc32[:, :, None], out=full)
    return full.reshape(F, 1, N, V)


# revision 37
# speedup vs baseline: 1.0058x; 1.0058x over previous
"""GNN message-passing attention kernel for Trainium2 (Bass/Tile).

Problem: 3 iterations of masked single-head attention over 1024 independent
graphs (N=256 nodes, V=40 features, QK=50).

Sharding: data-parallel on the leading F axis -- 128 graphs per NeuronCore
across 8 cores.  Weights replicated.  Full inputs in, full output out.

End-to-end wall time is dominated by the host<->device tunnel (~30 MB/s), so
the wire format is minimized:
  - values cross as int8 with a per-node f16 scale (10.5 + 0.5 MB), decoded
    to f32 on device (the ones-column the kernel carries is memset there);
  - adjacency crosses bit-packed along the neighbor axis (uint8, 8.4 MB) and
    is unpacked on-device with DVE shift+and, then cast to bf16;
  - the additive softmax mask is produced on the PE by transposing the
    unpacked adjacency against a MASKC-scaled identity directly into the
    score PSUM (no host-side transpose or bf16 cast);
  - the output crosses back as int8 with a per-node f16 scale (10.5 + 0.5 MB;
    the DVE int8 cast rounds to nearest, so quant error <= rowmax/252) and
    is dequantized on host.
Host prep overlaps the wire: values go to jax.device_put (async) before the
adjacency packbits runs, so the pack cost hides under the values transfer.
The PJRT shard_map callable is built once and cached (the library path
re-traces jax.jit and re-concatenates inputs on every call); the previous
call's device-resident output is donated back as the next call's output
buffer so no zero-buffer crosses the wire.

Device dataflow per 2-graph pipeline step (8 interleaved streams), as in the
proven baseline: q/k biases ride a ones-row through the weight matmuls
(fp32r), one Tanh per q|k PSUM block, scores accumulate on top of the
PE-transposed mask, one Exp with bias -1000 produces transposed numerators,
nv comes straight off num^T, and a per-partition reciprocal normalizes during
the PSUM->SBUF move (the ones-column refreshes itself for free).
"""

import math
import sys

import numpy as np

sys.path.insert(0, "/opt/trn_rl_repo")

import concourse.bass as bass  # noqa: E402
import concourse.mybir as mybir  # noqa: E402
from concourse import bacc, tile  # noqa: E402
from concourse.masks import make_identity  # noqa: E402

# Problem constants (hardcoded per harness contract).
F, N, V, QK = 1024, 256, 40, 50
ITERS = 3
SCALE = math.sqrt(50.0)  # NUM_QK = 50
MASKC = 1000.0 * SCALE  # adj * MASKC accumulated into e; exp bias -1000
N_CORES = 8
G = F // N_CORES  # graphs per core
NC2 = N // 128  # 2 partition chunks of the node axis
PB = N // 8  # packed bytes per adjacency row

F32 = mybir.dt.float32
F32R = mybir.dt.float32r  # fp32 data through the fast (replicated) PE path
F16 = mybir.dt.float16
BF16 = mybir.dt.bfloat16
U8 = mybir.dt.uint8
I8 = mybir.dt.int8
QMAX = 126.0  # int8 quant guard band (<127 so rounding can't wrap)

DEFAULT_BUFS = dict(io=10, work=10, small=11, vnb=22, vnhb=10, adjpb=6,
                    a8b=6, vnob=10, pmain=3, paux=2)


def build_nc(g_count=G, gb=2, streams=8, group=4, bufs=None):
    """Build the single-core Bass program (SPMD across 8 cores)."""
    B = dict(DEFAULT_BUFS)
    if bufs:
        B.update(bufs)
    streams = min(streams, g_count // gb)
    assert g_count % (gb * streams) == 0
    group = min(group, streams)
    nc = bacc.Bacc("TRN2", target_bir_lowering=False, debug=False)

    values_d = nc.dram_tensor("values", [g_count, N, V], I8, kind="ExternalInput")
    vscale_d = nc.dram_tensor("vscale", [g_count, N], F16, kind="ExternalInput")
    adjp_d = nc.dram_tensor("adjp", [g_count, N, PB], U8, kind="ExternalInput")
    wq_d = nc.dram_tensor("wq_aug", [V + 1, QK], F32R, kind="ExternalInput")
    wk_d = nc.dram_tensor("wk_aug", [V + 1, QK], F32R, kind="ExternalInput")
    out_d = nc.dram_tensor("out", [g_count, N, V], I8, kind="ExternalOutput")
    scales_d = nc.dram_tensor("scales", [g_count, N], F16, kind="ExternalOutput")

    with tile.TileContext(nc) as tc:
        with (
            tc.tile_pool(name="const", bufs=1) as constp,
            tc.tile_pool(name="io", bufs=B["io"]) as iop,
            tc.tile_pool(name="work", bufs=B["work"]) as workp,
            tc.tile_pool(name="small", bufs=B["small"]) as smallp,
            tc.tile_pool(name="pmain", bufs=B["pmain"], space="PSUM") as pmainp,
            tc.tile_pool(name="paux", bufs=B["paux"], space="PSUM") as pauxp,
        ):
            wq_sb = constp.tile([V + 1, QK], F32R)
            nc.sync.dma_start(wq_sb, wq_d[:, :])
            wk_sb = constp.tile([V + 1, QK], F32R)
            nc.sync.dma_start(wk_sb, wk_d[:, :])
            expbias_sb = constp.tile([128, 1], F32)
            nc.gpsimd.memset(expbias_sb, -1000.0)
            id_f32 = constp.tile([128, 128], F32)
            make_identity(nc, id_f32)
            # MASKC-scaled identity: moving operand that turns the unpacked
            # adjacency into the additive mask during the PE transpose.
            idm_mask = constp.tile([128, 128], BF16)
            nc.gpsimd.memset(idm_mask, 0.0)
            nc.gpsimd.affine_select(
                out=idm_mask, in_=idm_mask,
                compare_op=mybir.AluOpType.not_equal,
                fill=MASKC, base=0, pattern=[[-1, 128]], channel_multiplier=1,
            )

            class Stream:
                pass

            def phase_load(st, g0):
                st.prev_g0 = getattr(st, "g0", None)
                st.prev_vno = getattr(st, "vno", None)
                st.prev_sc = getattr(st, "sc", None)
                st.g0 = g0
                gsl = slice(g0, g0 + gb)
                st.vnh = iop.tile([128, gb, NC2, V], I8, tag="vnh",
                                  bufs=B["vnhb"])
                nc.sync.dma_start(
                    st.vnh,
                    values_d[gsl, :, :].rearrange("g (c p) v -> p g c v", c=NC2),
                )
                st.vsh = iop.tile([128, gb, NC2], F16, tag="vsh",
                                  bufs=B["vnhb"])
                nc.sync.dma_start(
                    st.vsh, vscale_d[gsl, :].rearrange("g (c p) -> p g c", c=NC2)
                )
                st.adjp = iop.tile([128, gb * NC2 * PB], U8, tag="adjp",
                                   bufs=B["adjpb"])
                nc.sync.dma_start(
                    st.adjp.rearrange("p (g c b) -> p g c b", g=gb, c=NC2),
                    adjp_d[gsl, :, :].rearrange("g (c p) b -> p g c b", c=NC2),
                )

            def phase_castv(st):
                vs32 = smallp.tile([128, gb, NC2], F32, tag="vs32")
                nc.vector.tensor_copy(vs32, st.vsh)
                st.vn = iop.tile([128, gb, NC2, V + 1], F32, tag="vn",
                                 bufs=B["vnb"])
                for g in range(gb):
                    for jc in range(NC2):
                        nc.vector.tensor_scalar_mul(
                            st.vn[:, g, jc, 0:V],
                            st.vnh[:, g, jc, :],
                            vs32[:, g, jc : jc + 1],
                        )
                nc.gpsimd.memset(st.vn[:, :, :, V : V + 1], 1.0)
                st.vnh = None
                st.vsh = None

            def phase_unpack(st):
                # adj[g, j, l] bits, l = 8*b + k (packbits MSB-first).
                a8 = smallp.tile([128, gb * NC2 * PB, 8], U8, tag="a8",
                                 bufs=B["a8b"])
                for k in range(8):
                    nc.vector.tensor_scalar(
                        a8[:, :, k], st.adjp, 7 - k, 1,
                        mybir.AluOpType.logical_shift_right,
                        mybir.AluOpType.bitwise_and,
                    )
                st.adjw = iop.tile([128, gb, NC2, N], BF16, tag="adjw")
                nc.vector.tensor_copy(
                    st.adjw.rearrange("p g c l -> p (g c l)"),
                    a8.rearrange("p x k -> p (x k)"),
                )
                st.adjp = None

            def phase_vt0(st):
                psum_vt = pauxp.tile([V + 1, gb * N], F32, tag="paux")
                for g in range(gb):
                    for c in range(NC2):
                        nc.tensor.transpose(
                            psum_vt[:, N * g + 128 * c : N * g + 128 * (c + 1)],
                            st.vn[:, g, c, :],
                            id_f32,
                        )
                st.vt = smallp.tile([V + 1, gb * N], F32R, tag="vt")
                nc.vector.tensor_copy(st.vt, psum_vt)

            def phase_qk(st):
                # [50, (qk-half, g, j)]: q in bank 0, k in bank 1.
                # Bias rides the vt ones-row (weights row V).
                st.psum_qk = pmainp.tile([QK, 2 * gb * N], F32, tag="pmain")
                nc.tensor.matmul(st.psum_qk[:, 0 : gb * N], wq_sb, st.vt)
                nc.tensor.matmul(st.psum_qk[:, gb * N : 2 * gb * N], wk_sb, st.vt)

            def phase_tanh(st):
                st.qk = workp.tile([QK, 2 * gb * N], F32R, tag="qk")
                nc.scalar.activation(
                    st.qk, st.psum_qk, mybir.ActivationFunctionType.Tanh
                )
                st.psum_qk = None

            def phase_et(st):
                st.psum_e = pmainp.tile([128, gb, NC2 * N], F32, tag="pmain", name="pe")
                # Scores init each 256-col region (start=True), then the two
                # 128-col mask transposes accumulate inside it -- the
                # start=True write must cover a superset of the start=False
                # writes or the scheduler can reorder them.
                for g in range(gb):
                    for lc in range(NC2):
                        nc.tensor.matmul(
                            st.psum_e[:, g, N * lc : N * (lc + 1)],
                            st.qk[:, gb * N + N * g + 128 * lc : gb * N + N * g + 128 * (lc + 1)],
                            st.qk[:, N * g : N * (g + 1)],
                            start=True,
                            stop=False,
                            skip_group_check=True,
                        )
                        for jc in range(NC2):
                            nc.tensor.matmul(
                                st.psum_e[:, g, N * lc + 128 * jc : N * lc + 128 * (jc + 1)],
                                st.adjw[:, g, jc, 128 * lc : 128 * (lc + 1)],
                                idm_mask,
                                start=False,
                                stop=(jc == NC2 - 1),
                                skip_group_check=True,
                            )

            def phase_exp(st):
                st.numt = workp.tile([128, gb, NC2 * N], F32, tag="numt")
                nc.scalar.activation(
                    st.numt,
                    st.psum_e,
                    mybir.ActivationFunctionType.Exp,
                    bias=expbias_sb,
                    scale=1.0 / SCALE,
                )
                st.psum_e = None

            def phase_nv(st):
                # nv[j, v] = sum_l num[j, l] v[l, v], directly off numT
                # (l already on partitions); the vn ones-column makes col V
                # the softmax row-sum.
                st.psum_nv = pauxp.tile([128, gb, NC2, V + 1], F32, tag="paux")
                for g in range(gb):
                    for jc in range(NC2):
                        for lc in range(NC2):
                            nc.tensor.matmul(
                                st.psum_nv[:, g, jc, :],
                                st.numt[:, g, N * lc + 128 * jc : N * lc + 128 * jc + 128],
                                st.vn[:, g, lc, :],
                                start=(lc == 0),
                                stop=(lc == NC2 - 1),
                            )
                st.numt = None

            def phase_norm(st, last):
                recip = smallp.tile([128, gb, NC2], F32, tag="recip")
                nc.vector.reciprocal(recip, st.psum_nv[:, :, :, V])
                if last:
                    # Final iteration: int8 row quantization straight off the
                    # unnormalized PSUM (the softmax recip folds into the
                    # scale, q = psum * QMAX/rowmax, v' = q * rowmax*recip/QMAX).
                    rm = smallp.tile([128, gb, NC2], F32, tag="rm")
                    nc.vector.tensor_reduce(
                        rm, st.psum_nv[:, :, :, 0:V],
                        axis=mybir.AxisListType.X, op=mybir.AluOpType.max,
                        apply_absolute_value=True,
                    )
                    qs = smallp.tile([128, gb, NC2], F32, tag="qs")
                    nc.vector.reciprocal(qs, rm)
                    st.sc = iop.tile([128, gb, NC2], F16, tag="sc",
                                     bufs=B["vnob"])
                    nc.vector.tensor_mul(st.sc, rm, recip)
                    st.vno = iop.tile([128, gb, NC2, V], I8, tag="vno",
                                      bufs=B["vnob"])
                    for g in range(gb):
                        for jc in range(NC2):
                            nc.vector.tensor_scalar(
                                st.vno[:, g, jc, :],
                                st.psum_nv[:, g, jc, 0:V],
                                qs[:, g, jc : jc + 1],
                                QMAX,
                                mybir.AluOpType.mult,
                                mybir.AluOpType.mult,
                            )
                else:
                    st.vn = iop.tile([128, gb, NC2, V + 1], F32, tag="vn",
                                     bufs=B["vnb"])
                    for g in range(gb):
                        for jc in range(NC2):
                            nc.vector.tensor_scalar_mul(
                                st.vn[:, g, jc, :],
                                st.psum_nv[:, g, jc, :],
                                recip[:, g, jc : jc + 1],
                            )
                st.psum_nv = None

            def phase_vt(st):
                psum_vt = pauxp.tile([V + 1, gb * N], F32, tag="paux")
                for g in range(gb):
                    for jc in range(NC2):
                        nc.tensor.transpose(
                            psum_vt[:, N * g + 128 * jc : N * g + 128 * (jc + 1)],
                            st.vn[:, g, jc, :],
                            id_f32,
                        )
                st.vt = smallp.tile([V + 1, gb * N], F32R, tag="vt")
                nc.vector.tensor_copy(st.vt, psum_vt)

            def phase_store_prev(st):
                # SWDGE (gpsimd) queue: keeps result stores out of the SP
                # FIFO so the next round's loads always prefetch early.
                gsl = slice(st.prev_g0, st.prev_g0 + gb)
                nc.gpsimd.dma_start(
                    out_d[gsl, :, :].rearrange("g (c p) v -> p g c v", c=NC2),
                    st.prev_vno,
                )
                nc.gpsimd.dma_start(
                    scales_d[gsl, :].rearrange("g (c p) -> p g c", c=NC2),
                    st.prev_sc,
                )

            sts = [Stream() for _ in range(streams)]
            for _i, _st in enumerate(sts):
                _st.sid = _i
            grps = [sts[i : i + group] for i in range(0, streams, group)]

            def run_iter(grp, t):
                for st in grp:
                    phase_qk(st)
                for st in grp:
                    phase_tanh(st)
                for st in grp:
                    phase_et(st)
                for st in grp:
                    phase_exp(st)
                for st in grp:
                    phase_nv(st)
                for st in grp:
                    phase_norm(st, t == ITERS - 1)
                if t < ITERS - 1:
                    for st in grp:
                        phase_vt(st)

            # Groups round-robin per iteration so one group's next phase
            # fills the pipeline while the other finishes; the previous
            # round's store and the next round's load ride inside the
            # rotation so round boundaries never resynchronize the streams.
            rounds = g_count // (gb * streams)
            for r in range(rounds):
                for grp in grps:
                    for st in grp:
                        phase_load(st, gb * (r * streams + st.sid))
                for grp in grps:
                    for st in grp:
                        if r > 0:
                            phase_store_prev(st)
                    for st in grp:
                        phase_castv(st)
                    for st in grp:
                        phase_unpack(st)
                    for st in grp:
                        phase_vt0(st)
                for t in range(ITERS):
                    for grp in grps:
                        run_iter(grp, t)
            for grp in grps:
                for st in grp:
                    st.prev_g0, st.prev_vno, st.prev_sc = st.g0, st.vno, st.sc
                    phase_store_prev(st)

    nc.compile()
    return nc


class _State:
    pass


_STATE = None


def _get_state():
    global _STATE
    if _STATE is None:
        import jax
        from jax.sharding import Mesh, NamedSharding, PartitionSpec
        from jax.experimental.shard_map import shard_map
        from concourse.bass2jax import (
            _bass_exec_p,
            install_neuronx_cc_hook,
            partition_id_tensor,
        )

        install_neuronx_cc_hook()
        nc = build_nc()
        assert nc.dbg_addr is None

        partition_name = (
            nc.partition_id_tensor.name if nc.partition_id_tensor else None
        )
        in_names, out_names, out_avals = [], [], []
        for alloc in nc.m.functions[0].allocations:
            if not isinstance(alloc, mybir.MemoryLocationSet):
                continue
            name = alloc.memorylocations[0].name
            if alloc.kind == "ExternalInput":
                if name != partition_name:
                    in_names.append(name)
            elif alloc.kind == "ExternalOutput":
                out_avals.append(
                    jax.core.ShapedArray(
                        tuple(alloc.tensor_shape), mybir.dt.np(alloc.dtype)
                    )
                )
                out_names.append(name)
        n_params = len(in_names)
        in_names_all = in_names + out_names
        if partition_name is not None:
            in_names_all.append(partition_name)

        def _body(*args):
            operands = list(args)
            if partition_name is not None:
                operands.append(partition_id_tensor())
            outs = _bass_exec_p.bind(
                *operands,
                out_avals=tuple(out_avals),
                in_names=tuple(in_names_all),
                out_names=tuple(out_names),
                lowering_input_output_aliases=(),
                sim_require_finite=True,
                sim_require_nnan=True,
                nc=nc,
            )
            return tuple(outs)

        devices = jax.devices()[:N_CORES]
        assert len(devices) == N_CORES
        mesh = Mesh(np.asarray(devices), ("core",))
        sharding = NamedSharding(mesh, PartitionSpec("core"))
        n_outs = len(out_names)
        donate = tuple(range(n_params, n_params + n_outs))
        sharded = jax.jit(
            shard_map(
                _body,
                mesh=mesh,
                in_specs=(PartitionSpec("core"),) * (n_params + n_outs),
                out_specs=(PartitionSpec("core"),) * n_outs,
                check_rep=False,
            ),
            donate_argnums=donate,
            keep_unused=True,
        )
        out_shapes = [
            ((N_CORES * a.shape[0], *a.shape[1:]), a.dtype) for a in out_avals
        ]

        def _zeros():
            import jax.numpy as jnp

            return tuple(jnp.zeros(s, d) for s, d in out_shapes)

        st = _State()
        st.jax = jax
        st.sharded = sharded
        st.in_names = in_names
        st.out_names = out_names
        st.sharding = sharding
        st.zeros_fn = jax.jit(_zeros, out_shardings=(sharding,) * n_outs)
        st.last_out = None
        _STATE = st
    return _STATE


def _aug(W, b):
    aug = np.zeros((V + 1, QK), np.float32)
    aug[0:V] = np.asarray(W, np.float32).T
    aug[V] = np.asarray(b, np.float32)
    return np.tile(aug, (N_CORES, 1))  # replicated across the core mesh


def _scratch(st):
    # Per-call numpy scratch, reused across calls (all device reads complete
    # before kernel() returns, so cross-call reuse cannot race a transfer).
    if not hasattr(st, "b_tmp"):
        st.b_tmp = np.empty((F, N, V), np.float32)
        st.b_s = np.empty((F, N), np.float32)
        st.b_s2 = np.empty((F, N), np.float32)
        st.b_vq = np.empty((F, N, V), np.int8)
        st.b_vs = np.empty((F, N), np.float16)
        st.b_a8 = np.empty((F, N, N), np.uint8)
    return st


def kernel(**inputs):
    st = _scratch(_get_state())
    # Stage values first (async H2D) so the adjacency packbits overlaps the
    # transfer's I/O waits (single-core host: the CPU share serializes).
    values = np.asarray(inputs["values"]).reshape(F, N, V)
    tmp, s = st.b_tmp, st.b_s
    # row abs-max via min+max (two reads, no 43MB abs write)
    s2 = st.b_s2
    np.max(values, axis=-1, out=s)
    np.min(values, axis=-1, out=s2)
    np.negative(s2, out=s2)
    np.maximum(s, s2, out=s)
    np.maximum(s, 1e-30, out=s)  # all-zero rows decode to 0 either way
    np.multiply(s, 1.0 / QMAX, out=s2)
    st.b_vs[...] = s2
    s_dev = st.jax.device_put(st.b_vs, st.sharding)
    np.divide(QMAX, s, out=s)
    np.multiply(values, s[:, :, None], out=tmp)
    np.rint(tmp, out=tmp)
    st.b_vq[...] = tmp
    v_dev = st.jax.device_put(st.b_vq, st.sharding)
    adj = np.asarray(inputs["adjacency_matrix"]).reshape(F, N, N)
    np.copyto(st.b_a8, adj, casting="unsafe")
    adjp = np.packbits(st.b_a8, axis=-1)
    a_dev = st.jax.device_put(adjp, st.sharding)
    wq_aug = _aug(inputs["Wq"], inputs["bq"])
    wk_aug = _aug(inputs["Wk"], inputs["bk"])
    wkey = hash((wq_aug.tobytes(), wk_aug.tobytes()))
    if getattr(st, "wkey", None) != wkey:
        st.wq_dev = st.jax.device_put(wq_aug, st.sharding)
        st.wk_dev = st.jax.device_put(wk_aug, st.sharding)
        st.wkey = wkey
    arrs = {
        "values": v_dev,
        "vscale": s_dev,
        "adjp": a_dev,
        "wq_aug": st.wq_dev,
        "wk_aug": st.wk_dev,
    }
    out_bufs = st.last_out if st.last_out is not None else st.zeros_fn()
    outs = st.sharded(*[arrs[n] for n in st.in_names], *out_bufs)
    st.last_out = outs  # device-resident; donated as next call's out buffers
    by_name = dict(zip(st.out_names, outs))
    by_name["out"].copy_to_host_async()
    by_name["scales"].copy_to_host_async()
    q = np.asarray(by_name["out"])  # (F, N, V) int8
    sc = np.asarray(by_name["scales"])  # (F, N) f16
    sc32 = st.b_s
    np.copyto(sc32, sc, casting="unsafe")
    np.multiply(sc32, 1.0 / QMAX, out=sc32)
    full = np.empty((F, N, V), np.float32)
    np.multiply(q, sc32[:, :, None], out=full)
    return full.reshape(F, 1, N, V)
